# revision 21
# baseline (speedup 1.0000x reference)
"""Trainium2 Bass kernel for nn_BoostedNeuralLDPCDecoder.

Weighted min-sum QC-LDPC decoder, 8 iterations.
  B=32, Z=384, N=68 VNs, M=46 CNs, dc=7, E=322 edges.

Strategy (pure data-parallel over batch, 4 batches per core on 8 cores):
  * All quantities kept in "2x units" (scaled by 2): post-quantize messages
    become integers in [-15, 15], so every sum (tot, v2c partials) is exact
    in f32 regardless of order -> bit-exact vs the f32 reference.
  * Messages live in the CN domain: SBUF tiles (92 partitions = (bsub,cn),
    free = (pos=7, z=384)).  CN min-sum runs entirely along the free dim.
  * The per-edge cyclic lift/unlift shifts are done by indirect DMA gathers
    (SWDGE) from doubled (2Z) scratch rows in DRAM: one index per
    (partition,pos) selects a 384-element contiguous run.
  * The VN scatter-add (tot) is a TensorEngine matmul over edge-partition
    tiles gathered into VN alignment.
  * v2c clip (+-20) is absorbed: unclipped magnitudes >= 40 (2x units)
    saturate the quantizer to 15 either way; selections are unaffected.
  * round-half-even via the f32 magic-number trick (x + 1.5*2^23) - 1.5*2^23.
"""

import os
import sys

import numpy as np

_TRN_REPO = "/opt/trn_rl_repo"
if _TRN_REPO not in sys.path:
    sys.path.insert(0, _TRN_REPO)

# Problem constants (hardcoded per the harness contract).
B, Z, N, M, DC, E, ITERS = 32, 384, 68, 46, 7, 322, 8
NCORES = 8
BL = B // NCORES          # batches per core = 4
Z2 = 2 * Z                # doubled-Z row length = 768
NT = 2                    # CN-domain tiles (bsub in {0,1} x 46 cns = 92 partitions)
PT = 2 * M                # partitions per CN tile = 92
ETILES = [128, 128, E - 256]   # edge-partition tile sizes for the VN matmul
MAGIC = 1.5 * (2.0 ** 23)      # f32 round-to-nearest-even magic constant
BIG = 1e30
QCLIP = 15.0                   # 2x of 7.5

_cache = {}


def _build_program(debug_dump_iter=None, skip_compile=False):
    """Build and compile the SPMD per-core bass program (shape-only; all
    graph data arrives via input tensors)."""
    import concourse.bass as bass
    import concourse.tile as tile
    from concourse import bacc, mybir

    f32 = mybir.dt.float32
    i32 = mybir.dt.int32
    Alu = mybir.AluOpType
    Act = mybir.ActivationFunctionType
    X = mybir.AxisListType.X

    nc = bacc.Bacc(
        "TRN2",
        target_bir_lowering=False,
        debug=False,
        enable_asserts=False,
        num_devices=NCORES,
    )

    # ---- external I/O ----
    xaD2 = nc.dram_tensor("xaD2", (1, BL * N * Z2), f32, kind="ExternalInput").ap()
    xaT2 = nc.dram_tensor("xaT2", (N, BL * Z), f32, kind="ExternalInput").ap()
    gidx_d = [
        nc.dram_tensor(f"gidx{t}", (PT, DC), i32, kind="ExternalInput").ap()
        for t in range(NT)
    ]
    vidx_d = [
        nc.dram_tensor(f"vidx{k}", (ETILES[k], BL), i32, kind="ExternalInput").ap()
        for k in range(3)
    ]
    V_d = [
        nc.dram_tensor(f"V{k}", (ETILES[k], N), f32, kind="ExternalInput").ap()
        for k in range(3)
    ]
    wneg_d = nc.dram_tensor("wneg", (PT, ITERS), f32, kind="ExternalInput").ap()
    out_d = nc.dram_tensor("out", (BL, N * Z), f32, kind="ExternalOutput").ap()
    if debug_dump_iter is not None:
        dbg_d = [
            nc.dram_tensor(f"dbg{t}", (PT, 6 * DC * Z), f32, kind="ExternalOutput").ap()
            for t in range(NT)
        ]

    with tile.TileContext(nc) as tc:
        with (
            tc.tile_pool(name="big", bufs=1) as bigp,
            tc.tile_pool(name="small", bufs=1) as smp,
            tc.tile_pool(name="psum", bufs=4, space="PSUM") as psp,
            tc.tile_pool(name="dram", bufs=1, space="DRAM") as drp,
        ):
            # ---- DRAM scratch (tracked pool tiles) ----
            msgD = drp.tile([1, BL * E * Z2], f32, tag="msgD", name="msgD")
            TOTd = drp.tile([1, BL * N * Z2], f32, tag="TOTd", name="TOTd")

            # ---- persistent SBUF tiles ----
            def cn_tiles(tag):
                return [
                    bigp.tile([PT, DC * Z], f32, tag=f"{tag}{t}", name=f"{tag}{t}") for t in range(NT)
                ]

            llr2 = cn_tiles("llr2")
            msgneg = cn_tiles("msgneg")
            buf = cn_tiles("buf")      # totE -> mt
            aab = cn_tiles("aab")      # |mt| -> ext values
            sgn = cn_tiles("sgn")      # sign(mt) -> rounded msg
            scr = cn_tiles("scr")      # llr2+totE -> masked -> msgpre
            i8 = mybir.dt.int8
            ismin = [
                bigp.tile([PT, DC * Z], i8, tag=f"ismin{t}", name=f"ismin{t}")
                for t in range(NT)
            ]

            def st_tiles(tag):
                return [bigp.tile([PT, Z], f32, tag=f"{tag}{t}", name=f"{tag}{t}") for t in range(NT)]

            min1 = st_tiles("min1")
            strict = st_tiles("strict")
            cnt = st_tiles("cnt")
            prod = st_tiles("prod")
            ge2 = [
                bigp.tile([PT, Z], mybir.dt.int8, tag=f"ge2{t}", name=f"ge2{t}")
                for t in range(NT)
            ]
            min2 = st_tiles("min2")
            m1s = st_tiles("m1s")
            m2s = st_tiles("m2s")
            st1 = st_tiles("st1")
            st2 = st_tiles("st2")
            pw = [bigp.tile([PT, 1], f32, tag=f"pw{t}", name=f"pw{t}") for t in range(NT)]

            gidx = [smp.tile([PT, DC], i32, tag=f"gidx{t}", name=f"gidxs{t}") for t in range(NT)]
            vidx = [smp.tile([ETILES[k], BL], i32, tag=f"vidx{k}", name=f"vidxs{k}") for k in range(3)]
            Vt = [smp.tile([ETILES[k], N], f32, tag=f"V{k}", name=f"Vs{k}") for k in range(3)]
            wneg = smp.tile([PT, ITERS], f32, tag="wneg", name="wnegs")
            c2v = [smp.tile([ETILES[k], BL * Z], f32, tag=f"c2v{k}", name=f"c2vs{k}") for k in range(3)]
            TOTsb = smp.tile([N, BL * Z], f32, tag="TOTsb", name="TOTsb")
            xaT2s = smp.tile([N, BL * Z], f32, tag="xaT2s", name="xaT2s")
            bias0 = smp.tile([PT, 1], f32, tag="bias0", name="bias0")

            # ---- init ----
            for t in range(NT):
                nc.sync.dma_start(gidx[t][:], gidx_d[t])
            for k in range(3):
                nc.sync.dma_start(vidx[k][:], vidx_d[k])
                nc.sync.dma_start(Vt[k][:], V_d[k])
            nc.sync.dma_start(wneg[:], wneg_d)
            nc.sync.dma_start(xaT2s[:], xaT2)
            nc.vector.memset(bias0[:], 1e-20)
            for t in range(NT):
                nc.vector.memset(msgneg[t][:], 0.0)
                # llr2 gather: llr2[p, pos, z] = xaD2[gidx[p,pos] + z]
                # (HW indirect DMA supports one index per partition per op)
                for pos in range(DC):
                    nc.gpsimd.indirect_dma_start(
                        out=llr2[t][:].rearrange("q (p z) -> q p z", p=DC)[:, pos, :],
                        out_offset=None,
                        in_=xaD2,
                        in_offset=bass.IndirectOffsetOnAxis(
                            ap=gidx[t][:, pos : pos + 1], axis=1
                        ),
                    )

            msgDv = msgD[:].rearrange(
                "o (b e r z) -> o b e r z", b=BL, e=E, r=2, z=Z
            )
            TOTdv = TOTd[:].rearrange(
                "o (b n r z) -> o b n r z", b=BL, n=N, r=2, z=Z
            )

            # ---- iterations ----
            for it in range(ITERS):
                for t in range(NT):
                    if it > 0:
                        # totE gather from doubled TOT rows
                        for pos in range(DC):
                            nc.gpsimd.indirect_dma_start(
                                out=buf[t][:].rearrange(
                                    "q (p z) -> q p z", p=DC
                                )[:, pos, :],
                                out_offset=None,
                                in_=TOTd[:],
                                in_offset=bass.IndirectOffsetOnAxis(
                                    ap=gidx[t][:, pos : pos + 1], axis=1
                                ),
                            )
                        # scr = llr2 + totE   (matches reference rounding order)
                        nc.vector.tensor_tensor(
                            out=scr[t][:], in0=llr2[t][:], in1=buf[t][:], op=Alu.add
                        )
                        # mt = scr + msgneg   (= llr + tot - c2v, 2x units)
                        nc.vector.scalar_tensor_tensor(
                            out=buf[t][:],
                            in0=msgneg[t][:],
                            scalar=1.0,
                            op0=Alu.bypass,
                            in1=scr[t][:],
                            op1=Alu.add,
                        )
                        mt = buf[t][:]
                    else:
                        mt = llr2[t][:]  # tot = c2v = 0 at iteration 0

                    mt3 = mt.rearrange("q (p z) -> q p z", p=DC)

                    # sign (exact, sign(0)=+1): s = 2*(mt>=0)-1 ; |mt| = mt*s
                    # (ACT Abs/Sign are LUT-based and not bit-exact)
                    nc.vector.tensor_scalar(
                        out=sgn[t][:], in0=mt, scalar1=0.0, scalar2=None,
                        op0=Alu.is_ge,
                    )
                    nc.vector.tensor_scalar(
                        out=sgn[t][:], in0=sgn[t][:], scalar1=2.0, scalar2=-1.0,
                        op0=Alu.mult, op1=Alu.add,
                    )
                    nc.vector.tensor_tensor(
                        out=aab[t][:], in0=mt, in1=sgn[t][:], op=Alu.mult
                    )

                    a3 = aab[t][:].rearrange("q (p z) -> q p z", p=DC)

                    # min1 = min over pos of |mt|  (tree of pairwise mins)
                    nc.vector.tensor_tensor(
                        out=min1[t][:], in0=a3[:, 0, :], in1=a3[:, 1, :], op=Alu.min
                    )
                    nc.vector.tensor_tensor(
                        out=strict[t][:], in0=a3[:, 2, :], in1=a3[:, 3, :], op=Alu.min
                    )
                    nc.vector.tensor_tensor(
                        out=cnt[t][:], in0=a3[:, 4, :], in1=a3[:, 5, :], op=Alu.min
                    )
                    nc.vector.tensor_tensor(
                        out=min1[t][:], in0=min1[t][:], in1=strict[t][:], op=Alu.min
                    )
                    nc.vector.tensor_tensor(
                        out=cnt[t][:], in0=cnt[t][:], in1=a3[:, 6, :], op=Alu.min
                    )
                    nc.vector.tensor_tensor(
                        out=min1[t][:], in0=min1[t][:], in1=cnt[t][:], op=Alu.min
                    )

                    min1b = min1[t][:].unsqueeze(1).broadcast_to([PT, DC, Z])

                    # ismin = (|mt| == min1)
                    nc.vector.tensor_tensor(
                        out=ismin[t][:].rearrange("q (p z) -> q p z", p=DC),
                        in0=a3,
                        in1=min1b,
                        op=Alu.is_equal,
                    )
                    # masked = |mt| + BIG*ismin
                    nc.vector.scalar_tensor_tensor(
                        out=scr[t][:],
                        in0=ismin[t][:],
                        scalar=BIG,
                        op0=Alu.mult,
                        in1=aab[t][:],
                        op1=Alu.add,
                    )
                    sc3 = scr[t][:].rearrange("q (p z) -> q p z", p=DC)
                    im3 = ismin[t][:].rearrange("q (p z) -> q p z", p=DC)
                    sg3 = sgn[t][:].rearrange("q (p z) -> q p z", p=DC)

                    # strict = min over pos of masked (2nd distinct value)
                    nc.vector.tensor_reduce(
                        out=strict[t][:], in_=sc3.transpose([0, 2, 1]),
                        axis=X, op=Alu.min,
                    )
                    # cnt = number of mins
                    nc.vector.tensor_reduce(
                        out=cnt[t][:], in_=im3.transpose([0, 2, 1]),
                        axis=X, op=Alu.add,
                    )
                    # prod = product of signs (pairwise tree; reduce-mult
                    # is not a supported DVE op)
                    nc.vector.tensor_tensor(
                        out=st1[t][:], in0=sg3[:, 0, :], in1=sg3[:, 1, :],
                        op=Alu.mult,
                    )
                    nc.vector.tensor_tensor(
                        out=st2[t][:], in0=sg3[:, 2, :], in1=sg3[:, 3, :],
                        op=Alu.mult,
                    )
                    nc.vector.tensor_tensor(
                        out=st1[t][:], in0=st1[t][:], in1=st2[t][:], op=Alu.mult
                    )
                    nc.vector.tensor_tensor(
                        out=st2[t][:], in0=sg3[:, 4, :], in1=sg3[:, 5, :],
                        op=Alu.mult,
                    )
                    nc.vector.tensor_tensor(
                        out=st2[t][:], in0=st2[t][:], in1=sg3[:, 6, :], op=Alu.mult
                    )
                    nc.vector.tensor_tensor(
                        out=prod[t][:], in0=st1[t][:], in1=st2[t][:], op=Alu.mult
                    )
                    # ge2 = (cnt >= 2) ; min2 = ge2 ? min1 : strict
                    nc.vector.tensor_scalar(
                        out=ge2[t][:], in0=cnt[t][:], scalar1=1.5, scalar2=None,
                        op0=Alu.is_ge,
                    )
                    nc.vector.tensor_copy(min2[t][:], strict[t][:])
                    nc.vector.copy_predicated(min2[t][:], ge2[t][:], min1[t][:])
                    # pw = -w[it] * prod ; m1s = min1*pw ; m2s = min2*pw
                    nc.vector.tensor_scalar(
                        out=pw[t][:], in0=wneg[:, it : it + 1], scalar1=1.0,
                        scalar2=None, op0=Alu.mult,
                    )
                    nc.vector.scalar_tensor_tensor(
                        out=m1s[t][:], in0=prod[t][:], scalar=pw[t][:],
                        op0=Alu.mult, in1=min1[t][:], op1=Alu.mult,
                    )
                    nc.vector.scalar_tensor_tensor(
                        out=m2s[t][:], in0=prod[t][:], scalar=pw[t][:],
                        op0=Alu.mult, in1=min2[t][:], op1=Alu.mult,
                    )

                    # ext (scaled by -w*prod): ismin ? m2s : m1s  -> aab
                    # exact select: broadcast-copy m1s, then overwrite the
                    # argmin positions with m2s (per-pos: keeps all operands
                    # plain 2-D so sim/HW agree; arithmetic select loses ulps)
                    nc.vector.tensor_copy(
                        aab[t][:].rearrange("q (p z) -> q p z", p=DC),
                        m1s[t][:].unsqueeze(1).broadcast_to([PT, DC, Z]),
                    )
                    for pos in range(DC):
                        nc.vector.copy_predicated(
                            aab[t][:].rearrange("q (p z) -> q p z", p=DC)[
                                :, pos, :
                            ],
                            im3[:, pos, :],
                            m2s[t][:],
                        )
                    # msgpre = ext * sign  (= -w * sgn_e * extmin)
                    nc.vector.tensor_tensor(
                        out=scr[t][:], in0=aab[t][:], in1=sgn[t][:], op=Alu.mult
                    )
                    # round-to-nearest-even, then clip to +-15 -> msgneg = -msg
                    nc.vector.tensor_scalar(
                        out=sgn[t][:], in0=scr[t][:], scalar1=MAGIC,
                        scalar2=MAGIC, op0=Alu.add, op1=Alu.subtract,
                    )
                    nc.vector.tensor_scalar(
                        out=msgneg[t][:], in0=sgn[t][:], scalar1=QCLIP,
                        scalar2=-QCLIP, op0=Alu.min, op1=Alu.max,
                    )

                    if debug_dump_iter == it:
                        dv = dbg_d[t].rearrange("q (s f) -> q s f", s=6)
                        nc.sync.dma_start(dv[:, 0, :], buf[t][:] if it > 0 else llr2[t][:])
                        nc.sync.dma_start(dv[:, 1, :], aab[t][:])
                        nc.sync.dma_start(dv[:, 2, :], sgn[t][:])
                        nc.sync.dma_start(dv[:, 3, :], scr[t][:])
                        nc.sync.dma_start(dv[:, 4, :], msgneg[t][:])
                        # stats: min1,strict,cnt,prod,min2 into slot 5
                        for j, stt in enumerate((min1, strict, cnt, prod, min2)):
                            nc.sync.dma_start(
                                dv[:, 5, j * Z : (j + 1) * Z], stt[t][:]
                            )
                    # write doubled msg rows to DRAM: msgD[b, e, r*Z+z]
                    src = msgneg[t][:].rearrange("q (p z) -> q p z", p=DC)
                    for r in range(2):
                        dst = (
                            msgDv[0, 2 * t : 2 * t + 2, :, r, :]
                            .rearrange("b (c p) z -> (b c) p z", c=M)
                        )
                        nc.sync.dma_start(dst, src)

                # VN-domain gather: c2v[k][e, b*Z+z] = msgD[vidx + z]
                for k in range(3):
                    for b in range(BL):
                        nc.gpsimd.indirect_dma_start(
                            out=c2v[k][:].rearrange("e (b z) -> e b z", b=BL)[
                                :, b, :
                            ],
                            out_offset=None,
                            in_=msgD[:],
                            in_offset=bass.IndirectOffsetOnAxis(
                                ap=vidx[k][:, b : b + 1], axis=1
                            ),
                        )

                # TOT' = sum over edges of msg (V entries are -1 ; c2v = -msg)
                for b in range(BL):
                    ps = psp.tile([N, Z], f32, tag="ps", name="ps")
                    for k in range(3):
                        nc.tensor.matmul(
                            ps[:],
                            lhsT=Vt[k][:],
                            rhs=c2v[k][:].rearrange("e (b z) -> e b z", b=BL)[
                                :, b, :
                            ],
                            start=(k == 0),
                            stop=(k == 2),
                        )
                    nc.scalar.copy(
                        TOTsb[:].rearrange("n (b z) -> n b z", b=BL)[:, b, :],
                        ps[:],
                    )

                if it < ITERS - 1:
                    # write doubled TOT rows: TOTd[b, n, r*Z+z]
                    srcT = TOTsb[:].rearrange("n (b z) -> n b z", b=BL)
                    for r in range(2):
                        dstT = TOTdv[0, :, :, r, :].transpose([1, 0, 2])
                        nc.sync.dma_start(dstT, srcT)
                else:
                    # out = (xa2 + tot') * 0.5, laid out (b, n*Z+z)
                    nc.vector.tensor_tensor(
                        out=TOTsb[:], in0=TOTsb[:], in1=xaT2s[:], op=Alu.add
                    )
                    nc.vector.tensor_scalar(
                        out=TOTsb[:], in0=TOTsb[:], scalar1=0.5, scalar2=None,
                        op0=Alu.mult,
                    )
                    nc.sync.dma_start(
                        out_d.rearrange("b (n z) -> n b z", n=N),
                        TOTsb[:].rearrange("n (b z) -> n b z", b=BL),
                    )

    if not skip_compile:
        nc.compile()
    return nc


def _host_inputs(xa, cn_weights, vn_idx, cn_idx, shifts):
    """Per-core input maps (host-side preprocessing)."""
    xa = np.asarray(xa, np.float32)
    cn_weights = np.asarray(cn_weights, np.float32)
    vn_idx = np.asarray(vn_idx).astype(np.int64)
    cn_idx = np.asarray(cn_idx).astype(np.int64)
    shifts = np.asarray(shifts).astype(np.int64)

    # Edge slots ordered CN-major (slot e' = cn*DC + pos).
    order = np.argsort(cn_idx, kind="stable")
    assert np.all(cn_idx[order] == np.repeat(np.arange(M), DC))
    nvec = vn_idx[order].astype(np.int64)     # VN per slot
    svec = shifts[order].astype(np.int64)     # shift per slot

    # CN-tile gather indices: p = bsub*M + cn ; b = 2t + bsub
    gidx = np.zeros((NT, PT, DC), np.int32)
    for t in range(NT):
        for bs in range(2):
            b = 2 * t + bs
            for cn in range(M):
                for pos in range(DC):
                    e = cn * DC + pos
                    gidx[t, bs * M + cn, pos] = (b * N + nvec[e]) * Z2 + svec[e]

    # VN-tile gather indices and (negated) incidence matrices
    vidx = []
    Vs = []
    estart = 0
    for k in range(3):
        P = ETILES[k]
        vi = np.zeros((P, BL), np.int32)
        Vk = np.zeros((P, N), np.float32)
        for r in range(P):
            e = estart + r
            for b in range(BL):
                vi[r, b] = (b * E + e) * Z2 + int((Z - svec[e]) % Z)
            Vk[r, nvec[e]] = -1.0
        vidx.append(vi)
        Vs.append(Vk)
        estart += P

    wneg = np.tile(-cn_weights[None, :ITERS], (PT, 1)).astype(np.float32)

    in_maps = []
    for c in range(NCORES):
        xs = xa[c * BL : (c + 1) * BL]          # (BL, Z, N)
        xt = 2.0 * np.transpose(xs, (2, 0, 1))  # (N, BL, Z), 2x units
        xaD2 = np.ascontiguousarray(
            np.concatenate([np.transpose(xt, (1, 0, 2))] * 2, axis=2)
        )  # (BL, N, 2Z)
        m = {
            "xaD2": xaD2.reshape(1, -1),
            "xaT2": np.ascontiguousarray(xt.reshape(N, BL * Z)),
            "wneg": wneg,
        }
        for t in range(NT):
            m[f"gidx{t}"] = gidx[t]
        for k in range(3):
            m[f"vidx{k}"] = vidx[k]
            m[f"V{k}"] = Vs[k]
        in_maps.append(m)
    return in_maps


def _ensure_ntff_hook():
    """The agent image's antenv lacks axon_hooks; synthesize it so
    trace=True can capture NTFF profiles via the axon .so."""
    import importlib
    import types

    try:
        importlib.import_module("antenv.axon_hooks")
        return
    except ImportError:
        pass
    try:
        import antenv
        from trn_agent_boot.trn_boot import _ntff_profile_via_ctypes

        mod = types.ModuleType("antenv.axon_hooks")
        _state = {"hook": _ntff_profile_via_ctypes("/opt/axon/libaxon_pjrt.so")}
        mod.get_axon_ntff_profile_hook = lambda: _state["hook"]
        mod.set_axon_ntff_profile_hook = lambda h: _state.__setitem__("hook", h)
        sys.modules["antenv.axon_hooks"] = mod
        antenv.axon_hooks = mod
    except Exception as e:  # degrade to no tracing
        print(f"ntff hook shim failed ({e}); tracing disabled", file=sys.stderr)


def kernel(xa, cn_weights, vn_idx, cn_idx, shifts, M=None, **_):
    from concourse.bass_utils import run_bass_kernel_spmd

    xa = np.asarray(xa)
    assert xa.shape == (B, Z, N), xa.shape

    if "nc" not in _cache:
        _cache["nc"] = _build_program()
    nc = _cache["nc"]

    in_maps = _host_inputs(xa, cn_weights, vn_idx, cn_idx, shifts)
    trace = bool(int(os.environ.get("KERNEL_TRACE", "0")))
    if trace:
        _ensure_ntff_hook()
    res = run_bass_kernel_spmd(
        nc, in_maps, core_ids=list(range(NCORES)), trace=trace
    )
    kernel._last_results = res

    out = np.concatenate([res.results[c]["out"] for c in range(NCORES)], axis=0)
    return out.astype(np.float32)


kernel._last_results = None
